# revision 8
# baseline (speedup 1.0000x reference)
"""Trainium2 Bass kernel for a 6-layer decoder (nn_Decoder_87488483820396).

Strategy: data-parallel over batch across the 8 NeuronCores. Cores 2b and
2b+1 both compute batch element b (weights replicated, no collectives); the
host takes the final hidden states and attention maps from one core of each
pair. The residual stream lives on-chip transposed: xT [D(part-tiles), S].
All matmuls run in bf16 with fp32 PSUM accumulation; layernorm statistics,
softmax normalisation of the output attention maps, and the residual stream
itself stay fp32.
"""

import numpy as np
import ml_dtypes

import concourse.bass as bass
import concourse.bacc as bacc
import concourse.tile as tile
from concourse import mybir
from concourse.masks import make_identity

P = 128
F32, BF16 = mybir.dt.float32, mybir.dt.bfloat16
AF = mybir.ActivationFunctionType
OP = mybir.AluOpType


class Cfg:
    def __init__(self, L=6, S=512, D=1024, H=16, DFF=4096, apply_gb=False):
        self.L, self.S, self.D, self.H, self.DFF = L, S, D, H, DFF
        self.DV = D // H
        self.SD, self.DD, self.FD = S // P, D // P, DFF // P
        self.apply_gb = apply_gb
        assert self.DV == 64 and D % 512 == 0 and DFF % 1024 == 0


def build_nc(cfg: Cfg):
    L, S, D, H, DFF = cfg.L, cfg.S, cfg.D, cfg.H, cfg.DFF
    DV, SD, DD, FD = cfg.DV, cfg.SD, cfg.DD, cfg.FD
    W1C = DFF // 1024          # W1 output-chunks of 1024 cols
    W2CW = D // 4              # W2 output-chunk width
    nc = bacc.Bacc()

    x0T = nc.dram_tensor("x0T", [D, S], F32, kind="ExternalInput").ap()
    maskm = nc.dram_tensor("maskm", [S, S], BF16, kind="ExternalInput").ap()
    wnames = ["wq1", "wk1", "wv1", "wo1", "wq2", "wk2", "wv2", "wo2"]
    wd = {n: nc.dram_tensor(n, [L, D, D], BF16, kind="ExternalInput").ap()
          for n in wnames}
    w1d = nc.dram_tensor("wff1", [L, D, DFF], BF16, kind="ExternalInput").ap()
    w2d = nc.dram_tensor("wff2", [L, DFF, D], BF16, kind="ExternalInput").ap()
    gbd = {}
    if cfg.apply_gb:
        for n in ["g1", "b1", "g2", "b2", "gff", "bff"]:
            gbd[n] = nc.dram_tensor(n, [L, D], F32, kind="ExternalInput").ap()
    x_out = nc.dram_tensor("x_out", [S, D], F32, kind="ExternalOutput").ap()
    attns_out = nc.dram_tensor("attns_out", [L, H, S, S], F32,
                               kind="ExternalOutput").ap()

    from contextlib import ExitStack

    with tile.TileContext(nc) as tc, ExitStack() as _ctx:
        pool_specs = [("const", 1), ("stream", 2), ("streamb", 1), ("qk", 1),
                      ("vp", 1), ("ep", 2), ("cp", 1), ("an", 3), ("hp", 1),
                      ("wp", 2), ("gbp", 2), ("sqp", 3), ("bcp", 1),
                      ("smol", 8), ("xo", 1)]
        psum_specs = [("psA", 3), ("psP", 2), ("psC", 2), ("psT", 1)]
        pools = {n: _ctx.enter_context(tc.tile_pool(name=n, bufs=b))
                 for n, b in pool_specs}
        dramp = _ctx.enter_context(tc.tile_pool(name="dramp", bufs=2,
                                                space="DRAM"))
        pools.update({n: _ctx.enter_context(
            tc.tile_pool(name=n, bufs=b, space="PSUM")) for n, b in psum_specs})
        constp, streamp, streambp, qkp, vp, ep, cp, anp, hp, wp, gbp, sqp, \
            bcp, smol, xop = (pools[n] for n, _ in pool_specs)
        psA, psP, psC, psT = (pools[n] for n, _ in psum_specs)

        ones128 = constp.tile([P, 1], BF16)
        nc.vector.memset(ones128[:], 1.0)
        ones64 = constp.tile([1, DV], BF16)
        nc.vector.memset(ones64[:], 1.0)
        identity = constp.tile([P, P], F32)
        make_identity(nc, identity[:])
        eps_sb = constp.tile([1, 1], F32)
        nc.vector.memset(eps_sb[:], 1e-5)
        mask_sb = constp.tile([P, SD, S], BF16)
        nc.sync.dma_start(out=mask_sb[:],
                          in_=maskm.rearrange("(kt p) q -> p kt q", p=P))

        def load_gb(name, l):
            if not cfg.apply_gb:
                return None
            t = gbp.tile([P, DD], F32, tag=name)
            nc.sync.dma_start(out=t[:],
                              in_=gbd[name][l].rearrange("(dd p) -> p dd", p=P))
            return t

        def layernorm(res, g_t, b_t):
            """In-place LN over the D (partition-tiled) axis of res
            [P, DD, S] f32; returns a fresh bf16 cast of the result."""
            psum_s = psC.tile([1, S], F32, tag="c")
            psum_q = psC.tile([1, S], F32, tag="c")
            for m in range(DD):
                rb16 = sqp.tile([P, S], BF16, tag="rb16")
                sq16 = sqp.tile([P, S], BF16, tag="sq16")
                nc.gpsimd.tensor_copy(out=rb16[:], in_=res[:, m, :])
                nc.scalar.activation(sq16[:], res[:, m, :], AF.Square)
                nc.tensor.matmul(psum_s[:], ones128[:], rb16[:],
                                 start=(m == 0), stop=(m == DD - 1))
                nc.tensor.matmul(psum_q[:], ones128[:], sq16[:],
                                 start=(m == 0), stop=(m == DD - 1))
            mean = smol.tile([1, S], F32, tag="st")
            nc.scalar.activation(mean[:], psum_s[:], AF.Copy, scale=1.0 / D)
            var = smol.tile([1, S], F32, tag="st")
            nc.scalar.activation(var[:], psum_q[:], AF.Copy, scale=1.0 / D)
            msq = smol.tile([1, S], F32, tag="st")
            nc.vector.tensor_mul(msq[:], mean[:], mean[:])
            nc.vector.tensor_tensor(var[:], var[:], msq[:], OP.subtract)
            sd_ = smol.tile([1, S], F32, tag="st")
            nc.scalar.activation(sd_[:], var[:], AF.Sqrt, bias=eps_sb[:])
            rstd = smol.tile([1, S], F32, tag="st")
            nc.vector.reciprocal(out=rstd[:], in_=sd_[:])
            mrs = smol.tile([1, S], F32, tag="st")
            nc.vector.tensor_mul(mrs[:], mean[:], rstd[:])
            rstd_b = bcp.tile([P, S], F32, tag="rstdb")
            mrs_b = bcp.tile([P, S], F32, tag="mrsb")
            dstat = dramp.tile([2, S], F32, tag="dstat")
            nc.sync.dma_start(out=dstat[0:1, :], in_=rstd[:])
            nc.sync.dma_start(out=dstat[1:2, :], in_=mrs[:])
            nc.sync.dma_start(out=rstd_b[:], in_=dstat[0:1, :].to_broadcast((P, S)))
            nc.sync.dma_start(out=mrs_b[:], in_=dstat[1:2, :].to_broadcast((P, S)))
            xTb = streambp.tile([P, DD, S], BF16, tag="xb")
            for m in range(DD):
                nc.vector.tensor_tensor(res[:, m, :], res[:, m, :], rstd_b[:],
                                        OP.mult)
                nc.vector.tensor_tensor(res[:, m, :], res[:, m, :], mrs_b[:],
                                        OP.subtract)
                if cfg.apply_gb:
                    nc.vector.tensor_scalar(
                        out=res[:, m, :], in0=res[:, m, :],
                        scalar1=g_t[:, m:m + 1], scalar2=b_t[:, m:m + 1],
                        op0=OP.mult, op1=OP.add)
                nc.gpsimd.tensor_copy(out=xTb[:, m, :], in_=res[:, m, :])
            return xTb

        def load_w(dram2d):
            w_sb = wp.tile([P, DD, D], BF16, tag="w")
            nc.sync.dma_start(out=w_sb[:],
                              in_=dram2d.rearrange("(kd p) n -> p kd n", p=P))
            return w_sb

        def mha(l, xT, xTb, wq_a, wk_a, wv_a, wo_a, g_t, b_t, masked, attn_out):
            wq_sb = load_w(wq_a[l])
            wk_sb = load_w(wk_a[l])
            wv_sb = load_w(wv_a[l])
            # Q^T, K^T: [dq (part, 2 heads/tile), tokens]
            qT = qkp.tile([P, DD, S], BF16, tag="qT")
            kT = qkp.tile([P, DD, S], BF16, tag="kT")
            for dst, w_sb in ((qT, wq_sb), (kT, wk_sb)):
                for m in range(DD):
                    ps = psP.tile([P, S], F32, tag="p")
                    for kd in range(DD):
                        nc.tensor.matmul(ps[:], w_sb[:, kd, m * P:(m + 1) * P],
                                         xTb[:, kd, :],
                                         start=(kd == 0), stop=(kd == DD - 1))
                    nc.scalar.activation(dst[:, m, :], ps[:], AF.Copy)
            # V natural [tokens (part), H, DV+1] with a ones column per head
            v_sb = vp.tile([P, SD, H, DV + 1], BF16, tag="v")
            for t in range(SD):
                for n2 in range(D // 512):
                    ps = psP.tile([P, 512], F32, tag="p")
                    for kd in range(DD):
                        nc.tensor.matmul(ps[:], xTb[:, kd, t * P:(t + 1) * P],
                                         wv_sb[:, kd, n2 * 512:(n2 + 1) * 512],
                                         start=(kd == 0), stop=(kd == DD - 1))
                    nc.vector.tensor_copy(
                        out=v_sb[:, t, n2 * 8:(n2 + 1) * 8, 0:DV],
                        in_=ps[:].rearrange("p (h d) -> p h d", h=8))
                nc.vector.memset(v_sb[:, t, :, DV:DV + 1], 1.0)
            # attention, head by head
            ctx = cp.tile([P, DD, S], BF16, tag="ctx")
            for h in range(H):
                m, r0 = h // 2, (h % 2) * DV
                expT = ep.tile([P, SD, S], BF16, tag="e")
                for kt in range(SD):
                    ps = psA.tile([P, S], F32, tag="a")
                    nc.tensor.matmul(ps[:], kT[r0:r0 + DV, m, kt * P:(kt + 1) * P],
                                     qT[r0:r0 + DV, m, :], start=True, stop=True)
                    nc.scalar.activation(expT[:, kt, :], ps[:], AF.Exp,
                                         scale=0.125)
                if masked:
                    nc.vector.tensor_mul(expT[:], expT[:], mask_sb[:])
                pc = psC.tile([DV + 1, S], F32, tag="c")
                for kt in range(SD):
                    nc.tensor.matmul(pc[:], v_sb[:, kt, h, :], expT[:, kt, :],
                                     start=(kt == 0), stop=(kt == SD - 1))
                rinv = smol.tile([1, S], BF16, tag="rinv")
                with nc.allow_low_precision(reason="softmax denom bcast in bf16"):
                    nc.vector.reciprocal(out=rinv[:], in_=pc[DV:DV + 1, :])
                pb = psC.tile([DV, S], F32, tag="c")
                nc.tensor.matmul(pb[:], ones64[:], rinv[:], start=True, stop=True)
                rb = smol.tile([DV, S], BF16, tag="rb")
                nc.scalar.activation(rb[:], pb[:], AF.Copy)
                nc.vector.tensor_tensor(ctx[r0:r0 + DV, m, :], pc[0:DV, :],
                                        rb[:], OP.mult)
                if attn_out:
                    # natural-orientation softmax for the attns output
                    for t in range(SD):
                        psn = psA.tile([P, S], F32, tag="a")
                        nc.tensor.matmul(psn[:],
                                         qT[r0:r0 + DV, m, t * P:(t + 1) * P],
                                         kT[r0:r0 + DV, m, :],
                                         start=True, stop=True)
                        an = anp.tile([P, S], F32, tag="an")
                        rs = smol.tile([P, 1], F32, tag="rs")
                        nc.scalar.activation(an[:], psn[:], AF.Exp, scale=0.125,
                                             accum_out=rs[:])
                        ri = smol.tile([P, 1], F32, tag="ri")
                        nc.vector.reciprocal(out=ri[:], in_=rs[:])
                        nc.vector.tensor_scalar_mul(out=an[:], in0=an[:],
                                                    scalar1=ri[:])
                        nc.sync.dma_start(
                            out=attns_out[l, h, t * P:(t + 1) * P, :], in_=an[:])
            # Wo projection + residual
            wo_sb = load_w(wo_a[l])
            res = streamp.tile([P, DD, S], F32, tag="xs")
            for m in range(DD):
                ps = psP.tile([P, S], F32, tag="p")
                for kh in range(DD):
                    nc.tensor.matmul(ps[:], wo_sb[:, kh, m * P:(m + 1) * P],
                                     ctx[:, kh, :],
                                     start=(kh == 0), stop=(kh == DD - 1))
                nc.vector.tensor_add(out=res[:, m, :], in0=ps[:],
                                     in1=xT[:, m, :])
            xTb2 = layernorm(res, g_t, b_t)
            return res, xTb2

        def ffn(l, xT, xTb, g_t, b_t):
            h_sb = hp.tile([P, FD, S], BF16, tag="h")
            w1r = w1d[l].rearrange("(kd p) f -> p kd f", p=P)
            for ci in range(W1C):
                w1c = wp.tile([P, DD, 1024], BF16, tag="w")
                nc.sync.dma_start(out=w1c[:],
                                  in_=w1r[:, :, ci * 1024:(ci + 1) * 1024])
                for mf in range(8):
                    ps = psP.tile([P, S], F32, tag="p")
                    for kd in range(DD):
                        nc.tensor.matmul(ps[:], w1c[:, kd, mf * P:(mf + 1) * P],
                                         xTb[:, kd, :],
                                         start=(kd == 0), stop=(kd == DD - 1))
                    nc.scalar.activation(h_sb[:, ci * 8 + mf, :], ps[:], AF.Relu)
            res = streamp.tile([P, DD, S], F32, tag="xs")
            w2r = w2d[l].rearrange("(kf p) d -> p kf d", p=P)
            for ci in range(4):
                w2c = wp.tile([P, FD, W2CW], BF16, tag="w")
                nc.sync.dma_start(out=w2c[:],
                                  in_=w2r[:, :, ci * W2CW:(ci + 1) * W2CW])
                for m2 in range(W2CW // P):
                    m = ci * (W2CW // P) + m2
                    ps = psP.tile([P, S], F32, tag="p")
                    for kf in range(FD):
                        nc.tensor.matmul(ps[:], w2c[:, kf, m2 * P:(m2 + 1) * P],
                                         h_sb[:, kf, :],
                                         start=(kf == 0), stop=(kf == FD - 1))
                    nc.vector.tensor_add(out=res[:, m, :], in0=ps[:],
                                         in1=xT[:, m, :])
            xTb2 = layernorm(res, g_t, b_t)
            return res, xTb2

        # ---- main ----
        xT = streamp.tile([P, DD, S], F32, tag="xs")
        nc.sync.dma_start(out=xT[:], in_=x0T.rearrange("(dd p) s -> p dd s", p=P))
        xTb = streambp.tile([P, DD, S], BF16, tag="xb")
        for m in range(DD):
            nc.vector.tensor_copy(out=xTb[:, m, :], in_=xT[:, m, :])
        for l in range(L):
            xT, xTb = mha(l, xT, xTb, wd["wq1"], wd["wk1"], wd["wv1"],
                          wd["wo1"], load_gb("g1", l), load_gb("b1", l),
                          masked=True, attn_out=False)
            xT, xTb = mha(l, xT, xTb, wd["wq2"], wd["wk2"], wd["wv2"],
                          wd["wo2"], load_gb("g2", l), load_gb("b2", l),
                          masked=False, attn_out=True)
            xT, xTb = ffn(l, xT, xTb, load_gb("gff", l), load_gb("bff", l))
        # final transpose back to [S, D]
        for t in range(SD):
            xo = xop.tile([P, D], F32, tag="xo")
            for m in range(DD):
                pt = psT.tile([P, P], F32, tag="t")
                nc.tensor.transpose(pt[:], xT[:, m, t * P:(t + 1) * P],
                                    identity[:])
                nc.vector.tensor_copy(out=xo[:, m * P:(m + 1) * P], in_=pt[:])
            nc.sync.dma_start(out=x_out[t * P:(t + 1) * P, :], in_=xo[:])

    nc.compile()
    return nc


def make_in_maps(cfg: Cfg, inputs, n_cores=8):
    """Host-side prep: embedding gather, mask build, bf16 weight casts."""
    bf = ml_dtypes.bfloat16
    dec = np.asarray(inputs["dec_inputs"])
    B = dec.shape[0]
    tok = np.asarray(inputs["tok_emb"], np.float32)
    pos = np.asarray(inputs["pos_emb"], np.float32)
    S = cfg.S
    x0 = tok[dec] + pos[None]                       # [B, S, D]
    x0T = np.ascontiguousarray(x0.transpose(0, 2, 1))  # [B, D, S]
    pad = (dec == 0)                                 # [B, S]
    look = np.triu(np.ones((S, S), bool), k=1)       # [q, k]
    masks = []
    for b in range(B):
        m = pad[b][None, :] | look                   # [q, k] True = masked
        masks.append(np.ascontiguousarray((~m).T.astype(bf)))  # [k, q] mult
    w16 = {n: np.ascontiguousarray(np.asarray(inputs[src], np.float32).astype(bf))
           for n, src in [("wq1", "Wq1"), ("wk1", "Wk1"), ("wv1", "Wv1"),
                          ("wo1", "Wo1"), ("wq2", "Wq2"), ("wk2", "Wk2"),
                          ("wv2", "Wv2"), ("wo2", "Wo2"), ("wff1", "Wff1"),
                          ("wff2", "Wff2")]}
    in_maps = []
    for c in range(n_cores):
        b = (c // 2) % B
        im = {"x0T": np.ascontiguousarray(x0T[b]), "maskm": masks[b]}
        im.update(w16)
        if cfg.apply_gb:
            for n, src in [("g1", "g1"), ("b1", "b1"), ("g2", "g2"),
                           ("b2", "b2"), ("gff", "gff"), ("bff", "bff")]:
                im[n] = np.asarray(inputs[src], np.float32)
        in_maps.append(im)
    return in_maps


_NC_CACHE = {}


def kernel(**inputs) -> tuple:
    from concourse.bass_utils import run_bass_kernel_spmd

    gb_arrs = [np.asarray(inputs[n]) for n in ["g1", "b1", "g2", "b2",
                                               "gff", "bff"]]
    apply_gb = not (all(np.all(a == 1.0) for a in gb_arrs[0::2])
                    and all(np.all(a == 0.0) for a in gb_arrs[1::2]))
    dec = np.asarray(inputs["dec_inputs"])
    B, S = dec.shape
    L, D = np.asarray(inputs["Wq1"]).shape[0], np.asarray(inputs["Wq1"]).shape[1]
    DFF = np.asarray(inputs["Wff1"]).shape[2]
    cfg = Cfg(L=L, S=S, D=D, H=16, DFF=DFF, apply_gb=apply_gb)

    key = (L, S, D, DFF, apply_gb)
    if key not in _NC_CACHE:
        _NC_CACHE[key] = build_nc(cfg)
    nc = _NC_CACHE[key]

    in_maps = make_in_maps(cfg, inputs, n_cores=8)
    out = run_bass_kernel_spmd(nc, in_maps, core_ids=list(range(8)))
    res = out.results

    H = cfg.H
    x = np.stack([res[2 * b]["x_out"] for b in range(B)])        # [B, S, D]
    attns = np.stack([res[2 * b]["attns_out"] for b in range(B)], axis=1)
    return x, attns                                               # [B,S,D], [L,B,H,S,S]
